# revision 1
# baseline (speedup 1.0000x reference)
"""ContextBlock (global-context attention pooling) Trainium2 kernel.

Reference computation (per sample b):
    scores[l] = sum_c w_c * x[c,l] + b          (1x1 conv C->1)
    attn      = softmax_L(scores)
    ctx[c]    = sum_l x[c,l] * attn[l]
    t         = relu(LN(w1 @ ctx + b1))         (LN over P=128)
    add[c]    = w2 @ t + b2
    y[c,l]    = x[c,l] + add[c]

Full shapes: x [16, 512, 8192] fp32. Data-parallel over batch across 8
cores (2 samples/core). Per sample, x (16 MiB) is kept SBUF-resident:
one read + one write of HBM (memory-bound target).

Implementation notes:
  - softmax without max-subtraction: scores ~ N(0, ~1) so exp() is safe
    in fp32; softmax(s + const) == softmax(s) so conv bias only shifts.
    1/sum(exp) is folded into ctx before the MLP.
  - scores via PE matmul (contract over channel partitions), exp on ACT
    with accum_out giving the running sum(exp) for free.
  - attn broadcast across partitions via K=1 ones-matmul into PSUM;
    ctx accumulated with DVE tensor_tensor_reduce (fused mul+reduce,
    chained through the `scalar` initial-value operand).
  - tiny MLP + LayerNorm on-chip with ones-matmul partition reductions.
  - y = x + add done in-place on the resident x tiles (DVE tensor_scalar
    with per-partition scalar), then DMA'd out.
"""

import numpy as np

import concourse.bass as bass
import concourse.bacc as bacc
import concourse.tile as tile
from concourse import mybir
from concourse import bass_utils

FP32 = mybir.dt.float32
AF = mybir.ActivationFunctionType
OP = mybir.AluOpType

B, C, L, P = 16, 512, 8192, 128
N_CORES = 8
B_LOC = B // N_CORES          # samples per core
G = C // 128                  # channel groups of 128 partitions
LB = 2048                     # L-block (columns resident per x tile)
NB = L // LB                  # blocks per sample
NCH = LB // 512               # 512-wide chunks per block (matmul N limit)
LN_EPS = 1e-5

# params_sb column layout
PW = 0            # conv_mask_w  [128, G]
PW1 = PW + G      # w1T arranged [128, G*128]
PW2 = PW1 + G * 128   # w2T     [128, C]
PLNW = PW2 + C    # ln_w [128,1]
PLNB = PLNW + 1   # ln_b [128,1]
PB1 = PLNB + 1    # b1   [128,1]
PB2 = PB1 + 1     # b2   [128, G]
PCOLS = PB2 + G


def _build_nc():
    nc = bacc.Bacc("TRN2", target_bir_lowering=False, debug=False)
    x_d = nc.dram_tensor("x", [B_LOC, C, L], FP32, kind="ExternalInput")
    y_d = nc.dram_tensor("y", [B_LOC, C, L], FP32, kind="ExternalOutput")
    p_d = nc.dram_tensor("params", [128, PCOLS], FP32, kind="ExternalInput")
    cb_d = nc.dram_tensor("convb", [1, 1], FP32, kind="ExternalInput")

    with tile.TileContext(nc) as tc:
        with (
            tc.tile_pool(name="xpool", bufs=5) as xpool,
            tc.tile_pool(name="singles", bufs=1) as singles,
            tc.tile_pool(name="tmp", bufs=2) as tmppool,
            tc.tile_pool(name="evec", bufs=2) as evecpool,
            tc.tile_pool(name="acc", bufs=2) as accpool,
            tc.tile_pool(name="small", bufs=2) as smallpool,
            tc.tile_pool(name="ps_s", bufs=2, space="PSUM") as ps_s,
            tc.tile_pool(name="ps_eb", bufs=1, space="PSUM") as ps_eb,
            tc.tile_pool(name="ps_sm", bufs=1, space="PSUM") as ps_sm,
            tc.tile_pool(name="ps_add", bufs=1, space="PSUM") as ps_add,
        ):
            params = singles.tile([128, PCOLS], FP32)
            nc.sync.dma_start(out=params, in_=p_d.ap())
            convb = singles.tile([1, 1], FP32)
            nc.sync.dma_start(out=convb, in_=cb_d.ap())
            # All-ones stationary matrix for partition-broadcast matmuls.
            # K=1 matmuls fault on HW ("matmuls with <128 partitions seems
            # to be problematic"), so broadcasts run with K=128 against rhs
            # tiles whose rows 1..127 are zeroed once at startup.
            ones_mat = singles.tile([128, 128], FP32)
            nc.vector.memset(ones_mat, 1.0)
            ones_col = singles.tile([128, 1], FP32)
            nc.vector.memset(ones_col, 1.0)
            eps_sb = singles.tile([1, 1], FP32)
            nc.vector.memset(eps_sb, LN_EPS)
            # rhs staging tiles for broadcasts: row 0 carries the payload.
            e_stage = singles.tile([128, LB], FP32)
            nc.vector.memset(e_stage, 0.0)
            sc_re = singles.tile([128, 1], FP32)
            nc.vector.memset(sc_re, 0.0)
            sc_mu = singles.tile([128, 1], FP32)
            nc.vector.memset(sc_mu, 0.0)
            sc_rsd = singles.tile([128, 1], FP32)
            nc.vector.memset(sc_rsd, 0.0)

            x_ap = x_d.ap().rearrange("b (g p) l -> b p g l", p=128)
            y_ap = y_d.ap().rearrange("b (g p) l -> b p g l", p=128)

            for b in range(B_LOC):
                x_tiles = []
                e_sums = evecpool.tile([1, NB * NCH], FP32, tag="esums")
                ctx_parts = accpool.tile([128, G * NB], FP32, tag="parts")
                ctx_acc = accpool.tile([128, G], FP32)

                for blk in range(NB):
                    xt = xpool.tile([128, G, LB], FP32, tag="xt")
                    x_tiles.append(xt)
                    nc.sync.dma_start(
                        out=xt, in_=x_ap[b, :, :, blk * LB:(blk + 1) * LB]
                    )
                    # scores for this block, 512 columns at a time
                    for ch in range(NCH):
                        s_ps = ps_s.tile([1, 512], FP32)
                        cs = slice(ch * 512, (ch + 1) * 512)
                        for g in range(G):
                            nc.tensor.matmul(
                                s_ps,
                                params[:, PW + g:PW + g + 1],
                                xt[:, g, cs],
                                start=(g == 0),
                                stop=(g == G - 1),
                            )
                        idx = blk * NCH + ch
                        nc.scalar.activation(
                            out=e_stage[0:1, cs],
                            in_=s_ps,
                            func=AF.Exp,
                            bias=convb[0:1, 0:1],
                            scale=1.0,
                            accum_out=e_sums[0:1, idx:idx + 1],
                        )
                    # broadcast exp(scores) to 128 partitions: K=128 matmul
                    # of all-ones against e_stage (rows 1..127 are zero)
                    e_b = ps_eb.tile([128, LB], FP32, tag="eb")
                    for ch in range(NCH):
                        cs = slice(ch * 512, (ch + 1) * 512)
                        nc.tensor.matmul(
                            e_b[:, cs],
                            ones_mat,
                            e_stage[:, cs],
                            start=True,
                            stop=True,
                        )
                    # ctx_parts[:, g*NB+blk] = sum_l x[:, g, l] * e_b[:, l]
                    # (tensor_tensor_reduce would fuse this but faults on HW)
                    for g in range(G):
                        tmp = tmppool.tile([128, LB], FP32, tag="tmp")
                        nc.vector.tensor_mul(tmp, xt[:, g, :], e_b)
                        nc.vector.tensor_reduce(
                            out=ctx_parts[:, g * NB + blk:g * NB + blk + 1],
                            in_=tmp, axis=mybir.AxisListType.X, op=OP.add,
                        )

                for g in range(G):
                    nc.vector.tensor_reduce(
                        out=ctx_acc[:, g:g + 1],
                        in_=ctx_parts[:, g * NB:(g + 1) * NB],
                        axis=mybir.AxisListType.X, op=OP.add,
                    )

                # ---- tail: softmax denom, MLP, LayerNorm ----
                s_e = smallpool.tile([1, 1], FP32)
                nc.vector.tensor_reduce(
                    out=s_e, in_=e_sums, axis=mybir.AxisListType.X, op=OP.add
                )
                nc.vector.reciprocal(out=sc_re[0:1, 0:1], in_=s_e)
                rvec_ps = ps_sm.tile([128, 1], FP32, tag="sm")
                nc.tensor.matmul(rvec_ps, ones_mat, sc_re, start=True, stop=True)
                rvec = smallpool.tile([128, 1], FP32)
                nc.scalar.copy(out=rvec, in_=rvec_ps)

                ctx_n = accpool.tile([128, G], FP32, tag="ctxn")
                for g in range(G):
                    nc.vector.tensor_scalar_mul(
                        ctx_n[:, g:g + 1], ctx_acc[:, g:g + 1], rvec[:, 0:1]
                    )

                # t = w1 @ ctx + b1
                t_ps = ps_sm.tile([128, 1], FP32, tag="sm")
                for g in range(G):
                    nc.tensor.matmul(
                        t_ps,
                        params[:, PW1 + g * 128:PW1 + (g + 1) * 128],
                        ctx_n[:, g:g + 1],
                        start=(g == 0),
                        stop=(g == G - 1),
                    )
                t_sb = smallpool.tile([128, 1], FP32)
                nc.scalar.activation(
                    out=t_sb, in_=t_ps, func=AF.Identity,
                    bias=params[:, PB1:PB1 + 1], scale=1.0,
                )
                # LayerNorm over the 128 partitions
                mu_ps = ps_sm.tile([1, 1], FP32, tag="sm")
                nc.tensor.matmul(mu_ps, t_sb, ones_col, start=True, stop=True)
                nc.scalar.mul(out=sc_mu[0:1, 0:1], in_=mu_ps, mul=1.0 / 128.0)
                muv_ps = ps_sm.tile([128, 1], FP32, tag="sm")
                nc.tensor.matmul(muv_ps, ones_mat, sc_mu, start=True, stop=True)
                d_sb = smallpool.tile([128, 1], FP32)
                nc.vector.tensor_sub(d_sb, t_sb, muv_ps)
                d2_sb = smallpool.tile([128, 1], FP32)
                nc.vector.tensor_mul(d2_sb, d_sb, d_sb)
                v_ps = ps_sm.tile([1, 1], FP32, tag="sm")
                nc.tensor.matmul(v_ps, d2_sb, ones_col, start=True, stop=True)
                sd_sb = smallpool.tile([1, 1], FP32)
                nc.scalar.activation(
                    out=sd_sb, in_=v_ps, func=AF.Sqrt,
                    bias=eps_sb[0:1, 0:1], scale=1.0 / 128.0,
                )
                nc.vector.reciprocal(out=sc_rsd[0:1, 0:1], in_=sd_sb)
                rsdv_ps = ps_sm.tile([128, 1], FP32, tag="sm")
                nc.tensor.matmul(rsdv_ps, ones_mat, sc_rsd, start=True, stop=True)
                rsdv_sb = smallpool.tile([128, 1], FP32)
                nc.scalar.copy(out=rsdv_sb, in_=rsdv_ps)
                h_sb = smallpool.tile([128, 1], FP32)
                nc.vector.tensor_scalar(
                    out=h_sb, in0=d_sb,
                    scalar1=rsdv_sb[:, 0:1], scalar2=params[:, PLNW:PLNW + 1],
                    op0=OP.mult, op1=OP.mult,
                )
                t_r = smallpool.tile([128, 1], FP32)
                nc.scalar.activation(
                    out=t_r, in_=h_sb, func=AF.Relu,
                    bias=params[:, PLNB:PLNB + 1], scale=1.0,
                )
                # add = w2 @ t_r + b2 (bank-aligned [128,1] PSUM per matmul)
                add_raw = accpool.tile([128, G], FP32, tag="addraw")
                for g in range(G):
                    a_ps = ps_add.tile([128, 1], FP32, tag="addps")
                    nc.tensor.matmul(
                        a_ps,
                        params[:, PW2 + g * 128:PW2 + (g + 1) * 128],
                        t_r,
                        start=True,
                        stop=True,
                    )
                    nc.scalar.copy(out=add_raw[:, g:g + 1], in_=a_ps)
                add_sb = accpool.tile([128, G], FP32, tag="addsb")
                nc.vector.tensor_add(add_sb, add_raw, params[:, PB2:PB2 + G])

                # y = x + add, in place, then store
                for blk in range(NB):
                    xt = x_tiles[blk]
                    for g in range(G):
                        nc.vector.tensor_scalar_add(
                            xt[:, g, :], xt[:, g, :], add_sb[:, g:g + 1]
                        )
                    nc.sync.dma_start(
                        out=y_ap[b, :, :, blk * LB:(blk + 1) * LB], in_=xt
                    )
    nc.compile()
    return nc


_NC_CACHE = None


def _get_nc():
    global _NC_CACHE
    if _NC_CACHE is None:
        _NC_CACHE = _build_nc()
    return _NC_CACHE


def _pack_params(conv_mask_w, w1, b1, ln_w, ln_b, w2, b2):
    p = np.zeros((128, PCOLS), dtype=np.float32)
    p[:, PW:PW + G] = conv_mask_w.reshape(G, 128).T
    # w1T[:, g*128+j] over partitions p  = w1[j, g*128+p]
    p[:, PW1:PW1 + G * 128] = (
        w1.T.reshape(G, 128, 128).transpose(1, 0, 2).reshape(128, G * 128)
    )
    p[:, PW2:PW2 + C] = w2.T
    p[:, PLNW] = ln_w
    p[:, PLNB] = ln_b
    p[:, PB1] = b1
    p[:, PB2:PB2 + G] = b2.reshape(G, 128).T
    return p


def kernel(x, conv_mask_w, conv_mask_b, w1, b1, ln_w, ln_b, w2, b2):
    x = np.ascontiguousarray(np.asarray(x, dtype=np.float32))
    params = _pack_params(
        np.asarray(conv_mask_w, np.float32), np.asarray(w1, np.float32),
        np.asarray(b1, np.float32), np.asarray(ln_w, np.float32),
        np.asarray(ln_b, np.float32), np.asarray(w2, np.float32),
        np.asarray(b2, np.float32),
    )
    convb = np.asarray(conv_mask_b, np.float32).reshape(1, 1)

    nc = _get_nc()
    in_maps = [
        {
            "x": np.ascontiguousarray(x[i * B_LOC:(i + 1) * B_LOC]),
            "params": params,
            "convb": convb,
        }
        for i in range(N_CORES)
    ]
    res = bass_utils.run_bass_kernel_spmd(
        nc, in_maps, core_ids=list(range(N_CORES))
    )
    return np.concatenate([r["y"] for r in res.results], axis=0)


if __name__ == "__main__":
    rng = np.random.default_rng(0)
    xs = {
        "x": rng.standard_normal((B, C, L), dtype=np.float32),
        "conv_mask_w": rng.standard_normal(C).astype(np.float32) / np.sqrt(C),
        "conv_mask_b": np.zeros(1, np.float32),
        "w1": rng.standard_normal((P, C)).astype(np.float32) / np.sqrt(C),
        "b1": np.zeros(P, np.float32),
        "ln_w": np.ones(P, np.float32),
        "ln_b": np.zeros(P, np.float32),
        "w2": rng.standard_normal((C, P)).astype(np.float32) / np.sqrt(P),
        "b2": np.zeros(C, np.float32),
    }
    y = kernel(**xs)
    print(y.shape, y.dtype)

